# revision 3
# baseline (speedup 1.0000x reference)
"""Bahdanau attention Trainium2 kernel.

Full-input contract: kernel(**inputs) takes the unsharded numpy inputs and
returns (context_vector, attention_weights) matching the fp32 reference.

Strategy: data-parallel over batch B=32 across 8 NeuronCores (4 batches per
core). Per core, the encoder tensor is streamed in twice in bf16 - once
transposed (E on partitions) for the projection matmul, once natural (S on
partitions) for the context matmul. Scores/softmax run in a batch-rows
layout (batch b on SBUF partition 32*b) so the whole softmax is a handful of
full-width DVE/ACT ops. Softmax skips max-subtraction: |scores| <= sum|v| so
exp cannot overflow in fp32.
"""

import numpy as np
import ml_dtypes

import concourse.bacc as bacc
import concourse.mybir as mybir
from concourse import tile
import concourse.bass as bass
from concourse.bass_utils import run_bass_kernel_spmd
from concourse.masks import make_identity

bf16 = ml_dtypes.bfloat16
F32 = mybir.dt.float32
BF16 = mybir.dt.bfloat16
I32 = mybir.dt.int32

B, S, E, D, A = 32, 2048, 1024, 1024, 256
NCORES = 8
NB = B // NCORES          # batches per core = 4
EC = E // 128             # e chunks = 8
SC4 = 4                   # s chunks of 512 for mm1/scores
SC16 = S // 128           # s chunks of 128 = 16
AC = A // 128             # a chunks = 2
ROW = 32                  # batch b lives on partition 32*b

# scores tile_position path: batch b's score row is written by the PE at
# output partition 32*b via tile_position=(0, 32b).
USE_TILE_POSITION = True


def build_nc():
    nc = bacc.Bacc("TRN2", target_bir_lowering=False, debug=False)

    encT = nc.dram_tensor("encT", (NB, E, S), BF16, kind="ExternalInput")
    encN = nc.dram_tensor("encN", (NB, S, E), BF16, kind="ExternalInput")
    wencT = nc.dram_tensor("wencT", (E, A), BF16, kind="ExternalInput")
    wdecT = nc.dram_tensor("wdecT", (D, A), BF16, kind="ExternalInput")
    hT = nc.dram_tensor("hT", (D, NB), BF16, kind="ExternalInput")
    vv = nc.dram_tensor("vv", (A, 1), BF16, kind="ExternalInput")
    lens = nc.dram_tensor("lens", (128, 1), I32, kind="ExternalInput")
    ctx_out = nc.dram_tensor("ctx_out", (NB, E), F32, kind="ExternalOutput")
    attn_out = nc.dram_tensor("attn_out", (NB, S), F32, kind="ExternalOutput")

    with tile.TileContext(nc) as tc:
        with (
            tc.tile_pool(name="consts", bufs=1) as consts,
            tc.tile_pool(name="smx", bufs=1) as smx,
            tc.tile_pool(name="encT_p", bufs=2) as encT_p,
            tc.tile_pool(name="encN_p", bufs=2) as encN_p,
            tc.tile_pool(name="tanh_p", bufs=2) as tanh_p,
            tc.tile_pool(name="mm1_ps", bufs=1, space="PSUM") as mm1_ps,
            tc.tile_pool(name="sc_ps", bufs=1, space="PSUM") as sc_ps,
            tc.tile_pool(name="ctx_ps", bufs=1, space="PSUM") as ctx_ps,
        ):
            # ---------- constants ----------
            w_sb = []
            for e in range(EC):
                w_t = consts.tile([128, A], BF16, name=f"w_t{e}")
                nc.sync.dma_start(w_t[:], wencT[e * 128:(e + 1) * 128, :])
                w_sb.append(w_t)
            wd_sb = []
            for d in range(EC):
                wd_t = consts.tile([128, A], BF16, name=f"wd_t{d}")
                nc.sync.dma_start(wd_t[:], wdecT[d * 128:(d + 1) * 128, :])
                wd_sb.append(wd_t)
            # hT (D, NB) -> (128, EC*NB): d-chunk d at cols [NB*d : NB*(d+1)]
            hT_sb = consts.tile([128, EC * NB], BF16)
            nc.sync.dma_start(
                hT_sb[:].rearrange("p (d b) -> p d b", d=EC),
                hT.rearrange("(d p) b -> p d b", d=EC),
            )
            # v (A, 1) -> (128, AC): a-chunk a at col a
            v_sb = consts.tile([128, AC], BF16)
            nc.sync.dma_start(
                v_sb[:].rearrange("p (a one) -> p a one", a=AC),
                vv.rearrange("(a p) one -> p a one", a=AC),
            )
            lens_sb = consts.tile([128, 1], I32)
            nc.sync.dma_start(lens_sb[:], lens[:])
            lensf = consts.tile([128, 1], F32)
            nc.vector.tensor_copy(lensf[:], lens_sb[:])
            ident = consts.tile([128, 128], F32)
            make_identity(nc, ident[:])

            # ---------- dec_proj = W_dec @ h  -> dec_sb (128, AC*NB) ----------
            dec_sb = consts.tile([128, AC * NB], F32)
            for a in range(AC):
                dps = mm1_ps.tile([128, 512], F32, name="dps", tag=f"ps{a}")
                for d in range(EC):
                    nc.tensor.matmul(
                        dps[:, :NB],
                        wd_sb[d][:, a * 128:(a + 1) * 128],
                        hT_sb[:, d * NB:(d + 1) * NB],
                        start=(d == 0), stop=(d == EC - 1),
                    )
                nc.vector.tensor_copy(dec_sb[:, a * NB:(a + 1) * NB], dps[:, :NB])

            # ---------- valid mask (batch-rows layout) ----------
            iota_t = smx.tile([128, S], F32, tag="scratchA")
            nc.gpsimd.iota(
                iota_t[:], pattern=[[1, S]], base=0, channel_multiplier=0,
                allow_small_or_imprecise_dtypes=True,
            )
            valid = smx.tile([128, S], F32)
            nc.vector.tensor_scalar(
                out=valid[:], in0=iota_t[:], scalar1=lensf[:], scalar2=None,
                op0=mybir.AluOpType.is_lt,
            )
            scores_sb = smx.tile([128, S], F32, tag="scratchB")
            nc.vector.memset(scores_sb[:], 0.0)

            # ---------- per-batch: enc_projT -> tanh -> scores ----------
            encN_tiles = {}
            for b in range(NB):
                et = encT_p.tile([128, EC * S], BF16, name=f"et{b}", tag="et")
                for e in range(EC):
                    nc.sync.dma_start(
                        et[:, e * S:(e + 1) * S],
                        encT[b, e * 128:(e + 1) * 128, :],
                    )
                # prefetch natural-layout tiles for phase 2 (b<2 fills bufs)
                en = encN_p.tile([128, SC16 * E], BF16, name=f"en{b}", tag="en")
                encN_tiles[b] = en
                for j in range(4):
                    nc.sync.dma_start(
                        en[:].rearrange("p (g e) -> p g e", g=SC16)[:, 4 * j:4 * (j + 1), :],
                        encN[b].rearrange("(g p) e -> p g e", g=SC16)[:, 4 * j:4 * (j + 1), :],
                    )

                tanh_tiles = []
                for a in range(AC):
                    tt = tanh_p.tile([128, S], BF16, name=f"tt{a}", tag=f"tt{a}")
                    for e in range(EC):
                        for sc in range(SC4):
                            if sc == 0 and e == 0:
                                pss = [mm1_ps.tile([128, 512], F32, name=f"ps{i}", tag=f"ps{i}") for i in range(SC4)]
                            nc.tensor.matmul(
                                pss[sc][:],
                                w_sb[e][:, a * 128:(a + 1) * 128],
                                et[:, e * S + sc * 512: e * S + (sc + 1) * 512],
                                start=(e == 0), stop=(e == EC - 1),
                            )
                    bias_ap = dec_sb[:, a * NB + b: a * NB + b + 1]
                    for sc in range(SC4):
                        nc.scalar.activation(
                            tt[:, sc * 512:(sc + 1) * 512], pss[sc][:],
                            mybir.ActivationFunctionType.Tanh,
                            bias=bias_ap, scale=1.0,
                        )
                    tanh_tiles.append(tt)

                # scores row for batch b -> scores_sb[32b, :]
                for sc in range(SC4):
                    sl = slice(sc * 512, (sc + 1) * 512)
                    if USE_TILE_POSITION:
                        sps = sc_ps.tile([128, 512], F32, tag="sps")
                        out_ap = sps[ROW * b: ROW * b + 1, :]
                        for a in range(AC):
                            nc.tensor.matmul(
                                out_ap,
                                v_sb[:, a: a + 1],
                                tanh_tiles[a][:, sl],
                                start=(a == 0), stop=(a == AC - 1),
                                tile_position=(0, ROW * b),
                            )
                        nc.scalar.copy(scores_sb[ROW * b: ROW * b + 1, sl], out_ap)
                    else:
                        sps = sc_ps.tile([1, 512], F32, tag="sps")
                        for a in range(AC):
                            nc.tensor.matmul(
                                sps[:],
                                v_sb[:, a: a + 1],
                                tanh_tiles[a][:, sl],
                                start=(a == 0), stop=(a == AC - 1),
                            )
                        stmp = smx.tile([1, 512], F32, tag="stmp")
                        nc.scalar.copy(stmp[:], sps[:])
                        nc.sync.dma_start(scores_sb[ROW * b: ROW * b + 1, sl], stmp[:])

            # ---------- softmax over all 4 batches at once ----------
            p_sb = smx.tile([128, S], F32, tag="scratchA")
            nc.scalar.activation(
                p_sb[:], scores_sb[:], mybir.ActivationFunctionType.Exp,
            )
            pm_sb = smx.tile([128, S], F32)
            nc.vector.tensor_tensor(
                out=pm_sb[:], in0=p_sb[:], in1=valid[:],
                op=mybir.AluOpType.mult,
            )
            sums = smx.tile([128, 1], F32)
            nc.vector.tensor_reduce(
                out=sums[:], in_=pm_sb[:], axis=mybir.AxisListType.X,
                op=mybir.AluOpType.add,
            )
            rec = smx.tile([128, 1], F32)
            nc.vector.reciprocal(rec[:], sums[:])
            attn_sb = smx.tile([128, S], F32, tag="scratchB")
            nc.vector.tensor_scalar(
                out=attn_sb[:], in0=pm_sb[:], scalar1=rec[:], scalar2=None,
                op0=mybir.AluOpType.mult,
            )
            for b in range(NB):
                nc.sync.dma_start(
                    attn_out[b: b + 1, :], attn_sb[ROW * b: ROW * b + 1, :]
                )

            # ---------- transpose attn to s-on-partitions (bf16) ----------
            attnT = smx.tile([128, S], BF16)
            for c in range(SC16):
                tp = mm1_ps.tile([128, 512], F32, name="tp", tag=f"ps{c % 4}")
                nc.tensor.transpose(
                    tp[:, :128], attn_sb[:, c * 128:(c + 1) * 128], ident[:]
                )
                nc.vector.tensor_copy(attnT[:, c * 128:(c + 1) * 128], tp[:, :128])

            # ---------- context: ctx[b] = attn[b] @ encN[b] ----------
            ctx_sb = smx.tile([128, E], F32)
            for b in range(NB):
                en = encN_tiles[b]
                for half in range(2):
                    if USE_TILE_POSITION:
                        cps = ctx_ps.tile([128, 512], F32, tag="cps")
                        out_ap = cps[ROW * b: ROW * b + 1, :]
                        tp_kw = dict(tile_position=(0, ROW * b))
                    else:
                        cps = ctx_ps.tile([1, 512], F32, tag="cps")
                        out_ap = cps[:]
                        tp_kw = {}
                    for sc in range(SC16):
                        nc.tensor.matmul(
                            out_ap,
                            attnT[:, sc * 128 + ROW * b: sc * 128 + ROW * b + 1],
                            en[:, sc * E + half * 512: sc * E + half * 512 + 512],
                            start=(sc == 0), stop=(sc == SC16 - 1),
                            **tp_kw,
                        )
                    if USE_TILE_POSITION:
                        nc.scalar.copy(
                            ctx_sb[ROW * b: ROW * b + 1, half * 512:(half + 1) * 512],
                            out_ap,
                        )
                    else:
                        ctmp = smx.tile([1, 512], F32, tag="ctmp")
                        nc.scalar.copy(ctmp[:], out_ap)
                        nc.sync.dma_start(
                            ctx_out[b: b + 1, half * 512:(half + 1) * 512], ctmp[:]
                        )
            if USE_TILE_POSITION:
                for b in range(NB):
                    nc.sync.dma_start(
                        ctx_out[b: b + 1, :], ctx_sb[ROW * b: ROW * b + 1, :]
                    )

    nc.compile()
    return nc


def make_in_maps(encoder_outputs, decoder_hidden, input_lengths, W_enc, W_dec, v):
    """Shard + lay out host-side. Returns list of per-core input dicts."""
    enc_b = encoder_outputs.astype(bf16)          # (B, S, E)
    encT_b = np.ascontiguousarray(enc_b.transpose(0, 2, 1))  # (B, E, S)
    wencT = np.ascontiguousarray(W_enc.T).astype(bf16)       # (E, A)
    wdecT = np.ascontiguousarray(W_dec.T).astype(bf16)       # (D, A)
    vvT = np.ascontiguousarray(v.reshape(1, A).T).astype(bf16)  # (A, 1)
    hT_all = decoder_hidden.T.astype(bf16)        # (D, B)

    in_maps = []
    for c in range(NCORES):
        sl = slice(c * NB, (c + 1) * NB)
        lens_exp = np.full((128, 1), S, dtype=np.int32)
        lens_exp[::ROW, 0][:NB] = input_lengths[sl]
        in_maps.append({
            "encT": np.ascontiguousarray(encT_b[sl]),
            "encN": np.ascontiguousarray(enc_b[sl]),
            "wencT": wencT,
            "wdecT": wdecT,
            "hT": np.ascontiguousarray(hT_all[:, sl]),
            "vv": vvT,
            "lens": lens_exp,
        })
    return in_maps


_NC_CACHE = None


def kernel(encoder_outputs, decoder_hidden, input_lengths, W_enc, W_dec, v):
    global _NC_CACHE
    if _NC_CACHE is None:
        _NC_CACHE = build_nc()
    nc = _NC_CACHE
    in_maps = make_in_maps(
        encoder_outputs, decoder_hidden, input_lengths, W_enc, W_dec, v
    )
    res = run_bass_kernel_spmd(nc, in_maps, core_ids=list(range(NCORES)))
    ctx = np.concatenate([r["ctx_out"] for r in res.results], axis=0)
    attn = np.concatenate([r["attn_out"] for r in res.results], axis=0)
    return ctx.astype(np.float32), attn.astype(np.float32)
